# revision 19
# baseline (speedup 1.0000x reference)
"""Bahdanau attention kernel for Trainium2 (8 NeuronCores, data-parallel over batch).

Reference computation (per batch row b):
    pq      = query @ Wq.T                       # (B, AD)
    hidden  = tanh(pq[:, None, :] + processed_memory)   # (B, T, AD)
    e       = einsum('btd,d->bt', hidden, v)     # (B, T)
    e       = where(mask, -1e30, e)
    out     = softmax(e, axis=1)

Sparsity: masked positions (mask==True, ~50% of T) contribute exactly 0 to the
softmax output and denominator (exp(-1e30) underflows to 0), so the host
compacts each batch row to its unmasked columns only (a gather is layout prep,
like the transpose the kernel already requires), padded to a fixed Tc.  The
device streams/tanhs/matmuls ~Tc=2176 columns instead of T=4096 — about half
the HBM traffic and half the ScalarE tanh work (the bottleneck engine:
1 elem/cycle/partition at 1.2 GHz, no fp16 discount).

Device strategy (per core, 8 batches):
  * compacted pm is host-laid-out [b, p, d, t] fp16 so each SBUF partition row
    is one contiguous 2*Tc = 8704B run (the DMA engines are rate-limited to
    ~11 B/ns each; fat contiguous rows keep them at peak).  The pm batches are
    split across BOTH hardware-dynamic DMA queues (sync + gpsimd) because one
    queue alone sustains only ~200 GB/s; gpsimd's DGE has a ~12us launch ramp
    so the first two batches ride the sync queue.
  * padding columns hold pm = -16*sign(v[d]), so every padded energy is
    ~ -sum|v| ~= -12.8 and exp() makes it ~3e-6: no keep-mask, no masked
    reduce, and the host scatter drops padded outputs anyway.
  * the per-d "+pq" add folds into the ScalarE tanh as a per-partition
    activation bias (free).  A dummy tanh at kernel start pulls the 1.3us
    ACT_TABLE_LOAD off the critical path.
  * energies accumulate into ONE [8, 2560] PSUM tile (5 banks): the stationary
    for batch b is v (x) e_b, a [128, 8] one-hot column matrix, so batch b's
    matmuls land in PSUM row b while other batches' matmuls add exact zeros
    there.  Matmuls trail each tanh immediately; PE cost is free-size bound,
    unchanged by M=8.  The last batch's tanh is split so its matmuls (and the
    tail) start earlier.
  * softmax runs directly on [8, Tc]: ONE exp reads the whole PSUM row
    (PSUM->SBUF fused) with accum_out producing the row sums, then
    reciprocal, and the 1/rowsum scale splits ScalarE (activation Copy with
    per-partition scale) || DVE.  Out rows are fat 8704B packets on the sync
    queue.
"""

import sys

if "/opt/trn_rl_repo" not in sys.path:
    sys.path.insert(0, "/opt/trn_rl_repo")

import numpy as np

import concourse.bacc as bacc
import concourse.bass as bass
import concourse.tile as tile
from concourse import mybir
from concourse.bass_utils import run_bass_kernel_spmd

B, T, QD, AD = 64, 4096, 1024, 256
NCORES = 8
BLOC = B // NCORES  # batches per core
KB = QD // 128      # k-blocks for the pq matmul
DB = AD // 128      # d-blocks (partition blocks of AD)
F32 = mybir.dt.float32
F16 = mybir.dt.float16
F8E3 = mybir.dt.float8e3



def build_nc(Tc: int) -> bass.Bass:
    chunks = []
    lo = 0
    while lo < Tc:
        chunks.append((lo, min(512, Tc - lo)))
        lo += 512
    PSW = -(-Tc // 512) * 512  # psum tile width, whole banks
    CW0 = 128 + BLOC
    V8O = DB * KB * CW0  # v8's offset inside wq_sb rows

    nc = bacc.Bacc(None, target_bir_lowering=False)

    pm2 = nc.declare_dram_parameter("pm2", [BLOC, 128, DB, Tc], F8E3, isOutput=False)
    # wqc[db, p, kb*(128+BLOC) + :] = [Wq cols | qT cols] per kb, and the d0
    # half additionally carries the flattened v8 at the row tail: one DMA per
    # d-half, all fat contiguous rows (small-packet DMAs starve their queue:
    # arbitration is per-packet round-robin across queues)
    wqc = nc.declare_dram_parameter(
        "wqc", [128, DB * KB * CW0 + DB * BLOC * 8], F16, isOutput=False
    )
    out = nc.declare_dram_parameter("out", [BLOC, Tc], F32, isOutput=True)

    Tanh = mybir.ActivationFunctionType.Tanh
    Exp = mybir.ActivationFunctionType.Exp
    Copy = mybir.ActivationFunctionType.Copy

    with tile.TileContext(nc) as tc:
        with (
            tc.tile_pool(name="singles", bufs=1) as singles,
            tc.tile_pool(name="pm", bufs=1) as pm_pool,
            tc.tile_pool(name="hid", bufs=3) as hid_pool,
            tc.tile_pool(name="energy", bufs=1, space="PSUM") as ep_pool,
            tc.tile_pool(name="spsum", bufs=2, space="PSUM") as sp_pool,
        ):
            # ---- act-table warm-up: dummy tanh so the 1.3us ACT_TABLE_LOAD
            # runs during startup DMA instead of gating the first real tanh
            zw = singles.tile([1, 1], F32)
            nc.gpsimd.memset(zw, 0)
            zw2 = singles.tile([1, 1], F32)
            nc.scalar.activation(out=zw2, in_=zw, func=Tanh, bias=zw[0:1, 0:1])

            # ---- pm tiles up front (bufs=8: no pool-reuse waits), DMAs
            # spread over all three hw-dynamic queues.  ScalarE issues two
            # during its idle startup window; batch 0 is split d0/d1 across
            # sync/gpsimd so the first tanh starts as early as possible.
            pm_tiles = []
            for b in range(BLOC):
                pmt = pm_pool.tile([128, DB, Tc], F8E3, name=f"pmt{b}")
                pm_tiles.append(pmt)
            # wq_sb flat [128, DB*KB*CW0 + 128]: per (d, kb), [128 wq cols |
            # BLOC qt cols]; flattened v8 at the row tail.  ONE fat-row DMA:
            # a queue's bandwidth share goes as its packet (row) size under
            # the per-packet round-robin arbitration.
            wq_sb = singles.tile([128, DB * KB * CW0 + DB * BLOC * 8], F16)

            # consumption-ordered 3-queue schedule; pm is fp8-e3m4 so the
            # whole stream is ~5.4MB — well inside the ~300+ GB/s the three
            # hw-dynamic queues sustain together
            # scalar queue: d0 halves of b0..b2 (ScalarE idle till the first
            # tanh; its DGE ring holds ~2 big DMAs — b2d0 is issued later,
            # inside the b0d0 wait window)
            nc.scalar.dma_start(out=pm_tiles[0][:, 0, :], in_=pm2[0][:, 0, :])
            nc.scalar.dma_start(out=pm_tiles[1][:, 0, :], in_=pm2[1][:, 0, :])

            # sync queue (live earliest): wqc then the late odd batches
            nc.sync.dma_start(out=wq_sb, in_=wqc[:, :])
            nc.sync.dma_start(out=pm_tiles[3], in_=pm2[3])
            nc.sync.dma_start(out=pm_tiles[5], in_=pm2[5])
            nc.sync.dma_start(out=pm_tiles[7], in_=pm2[7])

            # gpsimd queue: d1 halves of b0..b2, then b4, b6
            nc.gpsimd.dma_start(out=pm_tiles[0][:, 1, :], in_=pm2[0][:, 1, :])
            nc.gpsimd.dma_start(out=pm_tiles[1][:, 1, :], in_=pm2[1][:, 1, :])
            nc.gpsimd.dma_start(out=pm_tiles[2][:, 1, :], in_=pm2[2][:, 1, :])
            nc.gpsimd.dma_start(out=pm_tiles[4], in_=pm2[4])
            nc.gpsimd.dma_start(out=pm_tiles[6], in_=pm2[6])

            # ---- pq = Wq @ query.T, laid out [d % 128, dblk, b] ----
            pq_sb = singles.tile([128, DB, BLOC], F32)
            for d in range(DB):
                do = d * KB * CW0
                ppq = sp_pool.tile([128, BLOC], F32, tag="sp")
                for k in range(KB):
                    nc.tensor.matmul(
                        ppq,
                        lhsT=wq_sb[:, do + k * CW0 : do + k * CW0 + 128],
                        rhs=wq_sb[:, do + k * CW0 + 128 : do + (k + 1) * CW0],
                        start=(k == 0),
                        stop=(k == KB - 1),
                    )
                nc.vector.tensor_copy(out=pq_sb[:, d, :], in_=ppq)

            # ---- shared energies accumulator: one PSUM tile, 5 banks ----
            ep = ep_pool.tile([BLOC, PSW], F32)

            def emit_mms(b, d, lo_hi):
                first = b == 0 and d == 0
                last = b == BLOC - 1 and d == DB - 1
                for lo, w in chunks:
                    if lo < lo_hi[0] or lo >= lo_hi[1]:
                        continue
                    nc.tensor.matmul(
                        ep[:, lo : lo + w],
                        lhsT=wq_sb[:, V8O + (d * BLOC + b) * 8 : V8O + (d * BLOC + b) * 8 + 8],
                        rhs=h[:, d, lo : lo + w],
                        start=first,
                        stop=last,
                        skip_group_check=True,
                    )


            # b2d0's scalar-queue issue lands inside the b0d0 wait window
            nc.scalar.dma_start(out=pm_tiles[2][:, 0, :], in_=pm2[2][:, 0, :])

            # ---- main loop: tanh + one-hot v-reduction ----
            for b in range(BLOC):
                pm_sb = pm_tiles[b]
                h = hid_pool.tile([128, DB, Tc], F16)
                for d in range(DB):
                    last = b == BLOC - 1 and d == DB - 1
                    # split the very last tanh so its matmuls (and the whole
                    # softmax tail) start ~1.3us earlier
                    splits = [(0, 1536), (1536, Tc)] if last else [(0, Tc)]
                    for s0, s1 in splits:
                        nc.scalar.activation(
                            out=h[:, d, s0:s1],
                            in_=pm_sb[:, d, s0:s1],
                            func=Tanh,
                            bias=pq_sb[:, d, b : b + 1],
                            scale=1.0,
                        )
                        emit_mms(b, d, (s0, s1))

            # ---- softmax on [8, Tc]: one exp straight out of PSUM ----
            work = singles.tile([BLOC, Tc], F32)
            rsum = singles.tile([BLOC, 1], F32)
            nc.scalar.activation(
                out=work, in_=ep[:, 0:Tc], func=Exp, accum_out=rsum
            )
            rinv = singles.tile([BLOC, 1], F32)
            nc.vector.reciprocal(out=rinv, in_=rsum)

            # ---- scale by 1/rowsum, ScalarE || DVE quarters, pipelined out ----
            ow = singles.tile([BLOC, Tc], F32)
            SP = Tc // 2
            q0, q1 = SP // 2, SP + (Tc - SP) // 2
            # ScalarE quarters [0,q0), [q0,SP); DVE quarters [SP,q1), [q1,Tc)
            nc.scalar.activation(
                out=ow[:, 0:q0], in_=work[:, 0:q0], func=Copy, scale=rinv[:, 0:1]
            )
            nc.vector.tensor_scalar_mul(
                out=ow[:, SP:q1], in0=work[:, SP:q1], scalar1=rinv[:, 0:1]
            )
            nc.scalar.activation(
                out=ow[:, q0:SP], in_=work[:, q0:SP], func=Copy, scale=rinv[:, 0:1]
            )
            nc.vector.tensor_scalar_mul(
                out=ow[:, q1:Tc], in0=work[:, q1:Tc], scalar1=rinv[:, 0:1]
            )
            nc.sync.dma_start(out=out[:, :], in_=ow)

    nc.finalize()
    return nc


_CACHE: dict = {}


def _get_nc(key) -> bass.Bass:
    if key not in _CACHE:
        _CACHE[key] = build_nc(key)
    return _CACHE[key]


def _pick_tc(max_cnt: int) -> int:
    # fixed padded width, multiple of 128; 2176 covers the reference seed
    # (max count 2126) — recomputed per call so any mask works
    return max(2176, -(-(max_cnt + 1) // 128) * 128)


def make_in_maps(query, processed_memory, mask, Wq, v):
    query = np.ascontiguousarray(np.asarray(query, dtype=np.float32))
    pm = np.asarray(processed_memory, dtype=np.float32)
    mask_b = np.asarray(mask).astype(bool)
    Wq = np.asarray(Wq, dtype=np.float32)
    v = np.asarray(v, dtype=np.float32)

    keep = ~mask_b
    keep_idx = [np.flatnonzero(keep[gb]) for gb in range(B)]
    cnts = np.array([len(ix) for ix in keep_idx])
    Tc = _pick_tc(int(cnts.max()))
    key = Tc

    # WqR[db, p, kb, dl] = Wq[db*128+dl, kb*128+p]
    WqR = Wq.T.reshape(KB, 128, DB, 128).transpose(2, 1, 0, 3).astype(np.float16)
    v8 = np.zeros((128, DB * BLOC, 8), dtype=np.float16)
    for d in range(DB):
        for b in range(BLOC):
            v8[:, d * BLOC + b, b] = v[d * 128 : (d + 1) * 128]
    CW0 = 128 + BLOC
    # padding fill: tanh(pq - 8*sign(v)) ~= -sign(v), so padded energies are
    # ~ -sum|v| ~= -12.8 -> exp ~ 3e-6: negligible in the row sum, and the
    # host scatter drops padded outputs entirely
    from ml_dtypes import float8_e3m4
    padfill = (-8.0 * np.sign(v)).astype(float8_e3m4).reshape(DB, 128).T  # [128, DB]

    in_maps = []
    for i in range(NCORES):
        sl = slice(i * BLOC, (i + 1) * BLOC)
        pm2 = np.empty((BLOC, 128, DB, Tc), dtype=float8_e3m4)
        pm2[:, :, :, :] = padfill[None, :, :, None]
        for b in range(BLOC):
            gb = i * BLOC + b
            c = cnts[gb]
            # [c, AD] -> [AD, c] -> [DB, 128, c] -> [128, DB, c]
            pm2[b, :, :, :c] = (
                pm[gb, keep_idx[gb], :]
                .T.reshape(DB, 128, c)
                .transpose(1, 0, 2)
                .astype(float8_e3m4)
            )
        qt = (
            query[sl]
            .T.reshape(KB, 128, BLOC)
            .transpose(1, 0, 2)
            .astype(np.float16)
        )  # [128, KB, BLOC]
        wqc = np.empty((128, DB * KB * CW0 + DB * BLOC * 8), dtype=np.float16)
        for dd in range(DB):
            do = dd * KB * CW0
            for k in range(KB):
                wqc[:, do + k * CW0 : do + k * CW0 + 128] = WqR[dd, :, k, :]
                wqc[:, do + k * CW0 + 128 : do + (k + 1) * CW0] = qt[:, k, :]
        wqc[:, DB * KB * CW0 :] = v8.reshape(128, DB * BLOC * 8)
        in_maps.append(
            {
                "pm2": pm2,
                "wqc": np.ascontiguousarray(wqc),
            }
        )
    return in_maps, keep_idx, cnts, key


def run_spmd(in_maps, key=2176, **kwargs):
    return run_bass_kernel_spmd(_get_nc(key), in_maps, list(range(NCORES)), **kwargs)


def kernel(query, processed_memory, mask, Wq, v) -> np.ndarray:
    in_maps, keep_idx, cnts, key = make_in_maps(query, processed_memory, mask, Wq, v)
    res = run_spmd(in_maps, key=key)
    full = np.zeros((B, T), dtype=np.float32)
    for i in range(NCORES):
        outc = np.asarray(res.results[i]["out"], dtype=np.float32)
        for b in range(BLOC):
            gb = i * BLOC + b
            full[gb, keep_idx[gb]] = outc[b, : cnts[gb]]
    return full
